# revision 15
# baseline (speedup 1.0000x reference)
"""Trainium2 Bass kernel for nn_Node_GCN: out[n] = f(x[n]) + edge[n]^T @ g(cat(x,x)[n]).

Sharding: data-parallel over the batch dim N=8, one batch per NeuronCore.

edge is carried in fp8(e4m3) — halves the dominant HBM stream vs fp16 — and
gx is quantized to fp8 so the 2048x2048x128 edge contraction runs in
MatmulPerfMode.DoubleRow (216ns issue per 512-col matmul = the fp8 PE
roofline). Quantization error lands at rel_l2 ~1.8e-3 (vs 2e-2 budget).

DMA facts this schedule is built on (measured from ntff traces):
- All DGE rings share the 16 engines, but arbitration is per-packet, so a
  queue with small packets starves while the edge stream runs. Everything
  bandwidth-critical rides ONE ring (Sync), which round-robins across its
  in-flight DMAs at ~350-425 B/ns aggregate with 4KB packets.
- A DMA post costs ~0.7us of queue time; packets start ~0.8us later.
- The PE clock-gates to ~half speed unless continuously busy for several
  us (warm-up dummies bridge until the first data lands).

Order on the Sync ring: fb head (weights + xT half 0), bb, fb tail, then
the edge pairs smallest-granularity-last (pair 0 single, 1-4 as two 2-pair
DMAs, 5 and 6 single, pair 7 as 4 chunk DMAs so the final DoubleRow + bias
+ store chain hangs off 0.13MB). Output stores are two 1024-col DMAs (2KB
packets) on the same ring.

The device computes outT[n] = [h, j] fp16; the host transposes to [j, h]
and widens to fp32 while unsharding.
"""

import numpy as np

D_IN = 64
D_HID = 128
M = 2048          # nodes per batch
N_BATCH = 8
NCORES = 8

NT = M // 128     # 16 sender k-tiles
NPAIR = NT // 2   # 8 DoubleRow pairs
NCH = M // 512    # 4 output chunks of 512

# fp16 blob [128, FB_W]: weights first, then xT halves
_W_FW1 = 0          # f_w1 [64, 64]
_W_FW2 = 64         # f_w2 [64, 128]
_W_WG1 = 192        # wg1  [64, 128]  (= g_w1[:64] + g_w1[64:])
_W_GW2 = 320        # g_w2 [128, 128]
FW_W = 448
_F_XT0 = FW_W       # xT cols 0:512   (both 64-row halves)
_F_XT1 = FW_W + 512 # xT cols 512:1024
FB_W = FW_W + 1024

# fp32 bias blob [128, BB_W]
_B_GB2 = 0          # g_b2 [128, 128], broadcast 2x along free at use site
_B_F1 = 128         # f_b1 [64, 1]
_B_G1 = 129         # g_b1 [128, 1]
_B_F2 = 130         # f_b2 [128, 1]
BB_W = 131

N_DUMMIES = 16    # PE warm-up matmuls: the clock ramp needs ~4-5us of
                  # CONTINUOUS PE work before matmuls hit full rate, so the
                  # dummies run until the whole fb blob has landed and the
                  # MLP can proceed densely with no data stalls

_NC_CACHE = {}


def _build():
    import concourse.bacc as bacc
    import concourse.mybir as mybir
    from concourse.tile import TileContext
    from concourse.bass import ts

    f32 = mybir.dt.float32
    f16 = mybir.dt.float16
    f8 = mybir.dt.float8e4
    AF = mybir.ActivationFunctionType
    DR = mybir.MatmulPerfMode.DoubleRow

    nc = bacc.Bacc()
    fb_d = nc.declare_dram_parameter("fb", [128, FB_W], f16, isOutput=False)
    # partition-major fp8 edge: pairs 0-6 as [p, pair, t, j]; pair 7
    # chunk-major [p, c, t, j'] for the 4 tail DMAs
    edge_d = nc.declare_dram_parameter("edge", [128, NPAIR * 2 * M], f8, isOutput=False)
    bb_d = nc.declare_dram_parameter("bb", [128, BB_W], f32, isOutput=False)
    outT_d = nc.declare_dram_parameter("outT", [D_HID, M], f16, isOutput=True)

    with TileContext(nc) as tc:
        with (
            tc.tile_pool(name="const", bufs=1) as cpool,
            tc.tile_pool(name="acts", bufs=1) as apool,
            tc.tile_pool(name="edgep", bufs=NPAIR) as epool,
            tc.tile_pool(name="pout", bufs=1, space="PSUM") as pout_pool,
            tc.tile_pool(name="pg", bufs=2, space="PSUM") as pg_pool,
            tc.tile_pool(name="pwork", bufs=2, space="PSUM") as pwork_pool,
        ):
            fbh = cpool.tile([128, FW_W + 512], f16, name="fbh")  # weights + xT0
            fbx1 = cpool.tile([128, 512], f16, name="fbx1")       # xT1
            bb = cpool.tile([128, 1, BB_W], f32, name="bb")

            # ---- all input DMAs on the Sync ring. ed0 leads (the PE can't
            # start real work before the clock-ramp dummies finish ~11us
            # anyway, so fb arriving ~10.5 is free); then fb/bb, then the
            # rest of the edge stream.
            ed0 = epool.tile([128, 2, M], f8, tag="e", name="ed0")
            nc.sync.dma_start(out=ed0, in_=edge_d[:, 0:2 * M])
            nc.sync.dma_start(out=fbh, in_=fb_d[:, 0:FW_W + 512])
            nc.sync.dma_start(out=bb, in_=bb_d[:])
            nc.sync.dma_start(out=fbx1, in_=fb_d[:, FW_W + 512:FB_W])
            edd = [
                epool.tile([128, 2, 2, M], f8, tag="e", name=f"edd{d}")
                for d in range(2)
            ]
            for d in range(2):
                nc.sync.dma_start(
                    out=edd[d],
                    in_=edge_d[:, (1 + 2 * d) * 2 * M:(3 + 2 * d) * 2 * M],
                )
            ed5 = epool.tile([128, 2, M], f8, tag="e", name="ed5")
            nc.sync.dma_start(out=ed5, in_=edge_d[:, 5 * 2 * M:6 * 2 * M])
            ed6 = epool.tile([128, 2, M], f8, tag="e", name="ed6")
            nc.sync.dma_start(out=ed6, in_=edge_d[:, 6 * 2 * M:7 * 2 * M])
            eds = [ed0, edd[0][:, 0], edd[0][:, 1], edd[1][:, 0], edd[1][:, 1], ed5, ed6]
            ed7 = [
                epool.tile([128, 2, 512], f8, tag="e", name=f"ed7c{c}")
                for c in range(NCH)
            ]
            for c in range(NCH):
                nc.sync.dma_start(
                    out=ed7[c],
                    in_=edge_d[:, (NPAIR - 1) * 2 * M + c * 1024:(NPAIR - 1) * 2 * M + (c + 1) * 1024],
                )

            w_g2 = fbh[0:128, _W_GW2:_W_GW2 + 128]
            w_f2 = fbh[0:64, _W_FW2:_W_FW2 + 128]
            gb2b2 = bb[0:128, 0:1, _B_GB2:_B_GB2 + 128].broadcast_to([128, 2, 128])
            b_f1 = bb[0:64, 0, _B_F1:_B_F1 + 1]
            b_g1 = bb[0:128, 0, _B_G1:_B_G1 + 1]
            b_f2 = bb[0:128, 0, _B_F2:_B_F2 + 1]

            h1f = apool.tile([D_IN, M], f16, name="h1f")
            h1g = apool.tile([D_HID, M], f16, name="h1g")
            gx = apool.tile([128, NT, 128], f8, name="gx")  # tile i: [t, h] node-major
            # two store tiles: store A covers chunks 0-1, B covers 2-3, so
            # each 1024-col store DMA fires as soon as its two biases land
            outTab = [apool.tile([128, 1024], f16, name=f"outT{h}") for h in range(2)]
            pouts = [pout_pool.tile([128, 512], f32, name=f"pout{c}") for c in range(NCH)]

            # warm the ACT function table during the preamble (hoists the
            # lazy ~1.4us ACT_TABLE_LOAD off the h1g critical path)
            warm = apool.tile([1, 1], f32, name="warm")
            scratch = apool.tile([128, 256], f16, name="scratch")
            nc.gpsimd.memset(scratch, 0)
            nc.scalar.activation(warm, scratch[0:1, 0:1], AF.Relu, bias=0.0)

            # PE warm-up: the tensor engine clock-gates to ~half speed unless
            # continuously busy for several us; dummies bridge until fbh lands
            for _ in range(N_DUMMIES):
                psd = pg_pool.tile([128, 256], f32, tag="g", name="psd")
                nc.tensor.matmul(psd, scratch[:, 0:128], scratch, start=True, stop=True)

            # token-chunk c (tokens 512c..512c+512): xT half c%2, rows 64*(c//2)
            def tok_src(c):
                a, c2 = divmod(c, 2)
                src = fbh[:, _F_XT0:_F_XT0 + 512] if c2 == 0 else fbx1
                return src[64 * a:64 * a + 64, :]

            def rows_of(c):
                a = c // 2
                return slice(64 * a, 64 * a + 64)

            def h1g_chunk(c):
                rows = rows_of(c)
                w_g1 = fbh[rows, _W_WG1:_W_WG1 + 128]
                psg = pg_pool.tile([128, 512], f32, tag="g", name="psg")
                nc.tensor.matmul(psg, w_g1, tok_src(c), start=True, stop=True)
                nc.scalar.activation(h1g[:, ts(c, 512)], psg, AF.Relu, bias=b_g1)

            def h1f_chunk(c):
                # relu+bias on Scalar (GpSimd can't read PSUM; Vector stays
                # lean for the gx bias-adds on the DoubleRow critical path)
                rows = rows_of(c)
                w_f1 = fbh[rows, _W_FW1:_W_FW1 + 64]
                psf = pwork_pool.tile([64, 512], f32, tag="w", name="psf")
                nc.tensor.matmul(psf, w_f1, tok_src(c), start=True, stop=True)
                nc.scalar.activation(h1f[:, ts(c, 512)], psf, AF.Relu, bias=b_f1)

            def gx_pair(p):
                # gx tiles 2p, 2p+1 (node-major [t, h]): 2 matmuls into one
                # PSUM tile, one short DVE bias-add with fp8 downcast
                psx = pwork_pool.tile([128, 256], f32, tag="w", name="psx")
                for k in range(2):
                    i = 2 * p + k
                    nc.tensor.matmul(
                        psx[:, ts(k, 128)], h1g[:, ts(i, 128)], w_g2,
                        start=True, stop=True,
                    )
                nc.vector.tensor_add(gx[:, 2 * p:2 * p + 2, :], psx, gb2b2)

            def dr_pair(p, start=False, stop=False):
                lhsT = gx[:, 2 * p:2 * p + 2, :]
                for c in range(NCH):
                    rhs = ed7[c] if p == NPAIR - 1 else eds[p][:, :, ts(c, 512)]
                    nc.tensor.matmul(
                        pouts[c], lhsT, rhs,
                        start=start, stop=stop,
                        perf_mode=DR,
                    )

            # ---- dense MLP phase (all fb landed before the dummies end, so
            # no data stalls break the clock ramp), then the DoubleRow
            # stream paced by the edge DMA arrivals. The self-dyn opens fill
            # the h1g1-relu latency before gx pairs 2-3; gx pairs 6-7 slot
            # between the first DR pairs to cover the h1g3-relu latency.
            h1f_chunk(0)
            h1f_chunk(1)
            h1g_chunk(0)
            h1f_chunk(2)
            h1f_chunk(3)
            gx_pair(0)
            gx_pair(1)
            h1g_chunk(1)
            # self-dynamics opens each output chunk's PSUM accumulation group
            for c in range(NCH):
                nc.tensor.matmul(
                    pouts[c], w_f2, h1f[:, ts(c, 512)],
                    start=True, stop=False,
                )
            gx_pair(2)
            gx_pair(3)
            h1g_chunk(2)
            gx_pair(4)
            gx_pair(5)
            h1g_chunk(3)
            dr_pair(0)
            gx_pair(6)
            gx_pair(7)
            for p in range(1, NPAIR):
                dr_pair(p, stop=(p == NPAIR - 1))

            # tail: per-chunk bias-add split into ACT/DVE halves (parallel,
            # halves the bias latency); two 1024-col stores on the Sync ring
            # (2KB packets move ~2-3x faster than 1KB ones)
            for c in range(NCH):
                dst = outTab[c // 2][:, ts(c % 2, 512)]
                nc.scalar.activation(
                    dst[:, 0:256], pouts[c][:, 0:256], AF.Identity, bias=b_f2
                )
                nc.vector.tensor_scalar_add(
                    dst[:, 256:512], pouts[c][:, 256:512], b_f2
                )
                if c % 2 == 1:
                    nc.sync.dma_start(
                        out=outT_d[:, ts(c // 2, 1024)], in_=outTab[c // 2]
                    )
    nc.compile()
    return nc


def _get_nc():
    if "nc" not in _NC_CACHE:
        _NC_CACHE["nc"] = _build()
    return _NC_CACHE["nc"]


def _prep_in_maps(inputs):
    import ml_dtypes

    x = np.asarray(inputs["x"], dtype=np.float32)
    edge = np.asarray(inputs["edge"], dtype=np.float32)
    f_w1 = np.asarray(inputs["f_w1"], dtype=np.float32)
    f_b1 = np.asarray(inputs["f_b1"], dtype=np.float32)
    f_w2 = np.asarray(inputs["f_w2"], dtype=np.float32)
    f_b2 = np.asarray(inputs["f_b2"], dtype=np.float32)
    g_w1 = np.asarray(inputs["g_w1"], dtype=np.float32)
    g_b1 = np.asarray(inputs["g_b1"], dtype=np.float32)
    g_w2 = np.asarray(inputs["g_w2"], dtype=np.float32)
    g_b2 = np.asarray(inputs["g_b2"], dtype=np.float32)

    # cat(x, x) @ g_w1 == x @ (g_w1[:64] + g_w1[64:])
    wg1 = g_w1[:D_IN] + g_w1[D_IN:]

    # x[n].T packed [128, 1024]: xT2[64a + k, t] = x[n, 1024a + t, k]
    xT = np.transpose(x, (0, 2, 1)).astype(np.float16)       # [8, 64, 2048]
    xT2 = np.concatenate([xT[:, :, :1024], xT[:, :, 1024:]], axis=1)  # [8, 128, 1024]

    fb = np.zeros((N_BATCH, 128, FB_W), dtype=np.float16)
    for r in (slice(0, 64), slice(64, 128)):  # duplicate for partition-64 rhs
        fb[:, r, _W_FW1:_W_FW1 + 64] = f_w1.astype(np.float16)
        fb[:, r, _W_FW2:_W_FW2 + 128] = f_w2.astype(np.float16)
        fb[:, r, _W_WG1:_W_WG1 + 128] = wg1.astype(np.float16)
    fb[:, 0:128, _W_GW2:_W_GW2 + 128] = g_w2.astype(np.float16)
    fb[:, :, _F_XT0:_F_XT0 + 512] = xT2[:, :, 0:512]
    fb[:, :, _F_XT1:_F_XT1 + 512] = xT2[:, :, 512:1024]
    fb = np.ascontiguousarray(fb)

    bb = np.zeros((128, BB_W), dtype=np.float32)
    bb[0:128, _B_GB2:_B_GB2 + 128] = g_b2[None, :]
    bb[0:64, _B_F1] = f_b1
    bb[0:128, _B_G1] = g_b1
    bb[0:128, _B_F2] = f_b2

    # edge fp8 partition-major: edge_r[n][p, pair, t, j] = edge8[n, 128*(2*pair+t)+p, j]
    # except pair 7, which is chunk-major [p, c, t, j'] for the 4 tail DMAs
    edge8 = edge.astype(ml_dtypes.float8_e4m3)               # [8, 2048, 2048]
    edge_r = (
        edge8.reshape(N_BATCH, NPAIR, 2, 128, M)
        .transpose(0, 3, 1, 2, 4)
        .reshape(N_BATCH, 128, NPAIR, 2 * M)
        .copy()
    )
    p7 = (
        edge8[:, (NPAIR - 1) * 256:, :]                      # [8, 256, 2048]
        .reshape(N_BATCH, 2, 128, NCH, 512)
        .transpose(0, 2, 3, 1, 4)                            # [8, 128, c, t, 512]
        .reshape(N_BATCH, 128, 2 * M)
    )
    edge_r[:, :, NPAIR - 1, :] = p7
    edge_r = np.ascontiguousarray(edge_r).reshape(N_BATCH, 128, NPAIR * 2 * M)

    in_maps = [
        {
            "fb": fb[n],
            "edge": edge_r[n],
            "bb": bb,
        }
        for n in range(N_BATCH)
    ]
    return in_maps


def run(inputs, trace=False, **kw):
    """Run on 8 cores; returns (out [8, 2048, 128] fp32, BassKernelResults)."""
    from concourse.bass_utils import run_bass_kernel_spmd

    nc = _get_nc()
    in_maps = _prep_in_maps(inputs)
    res = run_bass_kernel_spmd(nc, in_maps, list(range(NCORES)), trace=trace, **kw)
    outT = np.stack([np.asarray(res.results[n]["outT"]) for n in range(N_BATCH)])
    out = np.ascontiguousarray(np.transpose(outT, (0, 2, 1)))  # [8, 2048, 128]
    return out.astype(np.float32), res


def kernel(**inputs):
    out, _ = run(inputs, trace=False)
    return out


# revision 17
# speedup vs baseline: 1.0687x; 1.0687x over previous
"""Trainium2 Bass kernel for nn_Node_GCN: out[n] = f(x[n]) + edge[n]^T @ g(cat(x,x)[n]).

Sharding: data-parallel over the batch dim N=8, one batch per NeuronCore.

edge is carried in fp8(e4m3) — halves the dominant HBM stream vs fp16 — and
gx is quantized to fp8 so the 2048x2048x128 edge contraction runs in
MatmulPerfMode.DoubleRow (216ns issue per 512-col matmul = the fp8 PE
roofline). Quantization error lands at rel_l2 ~1.8e-3 (vs 2e-2 budget).

DMA facts this schedule is built on (measured from ntff traces):
- All DGE rings share the 16 engines, but arbitration is per-packet, so a
  queue with small packets starves while the edge stream runs. Everything
  bandwidth-critical rides ONE ring (Sync), which round-robins across its
  in-flight DMAs at ~350-425 B/ns aggregate with 4KB packets.
- A DMA post costs ~0.7us of queue time; packets start ~0.8us later.
- The PE clock-gates to ~half speed unless continuously busy for several
  us (warm-up dummies bridge until the first data lands).

Order on the Sync ring: fb head (weights + xT half 0), bb, fb tail, then
the edge pairs smallest-granularity-last (pair 0 single, 1-4 as two 2-pair
DMAs, 5 and 6 single, pair 7 as 4 chunk DMAs so the final DoubleRow + bias
+ store chain hangs off 0.13MB). Output stores are two 1024-col DMAs (2KB
packets) on the same ring.

The device computes outT[n] = [h, j] fp16; the host transposes to [j, h]
and widens to fp32 while unsharding.
"""

import numpy as np

D_IN = 64
D_HID = 128
M = 2048          # nodes per batch
N_BATCH = 8
NCORES = 8

NT = M // 128     # 16 sender k-tiles
NPAIR = NT // 2   # 8 DoubleRow pairs
NCH = M // 512    # 4 output chunks of 512

# fp16 blob [128, FB_W]: weights first, then xT halves
_W_FW1 = 0          # f_w1 [64, 64]
_W_FW2 = 64         # f_w2 [64, 128]
_W_WG1 = 192        # wg1  [64, 128]  (= g_w1[:64] + g_w1[64:])
_W_GW2 = 320        # g_w2 [128, 128]
FW_W = 448
_F_XT0 = FW_W       # xT cols 0:512   (both 64-row halves)
_F_XT1 = FW_W + 512 # xT cols 512:1024
FB_W = FW_W + 1024

# fp32 bias blob [128, BB_W]
_B_GB2 = 0          # g_b2 [128, 128], broadcast 2x along free at use site
_B_F1 = 128         # f_b1 [64, 1]
_B_G1 = 129         # g_b1 [128, 1]
_B_F2 = 130         # f_b2 [128, 1]
BB_W = 131

N_DUMMIES = 12    # PE warm-up matmuls (213ns each): sized to end just after
                  # the fb blob lands (~9.3-9.6us) with margin — a PE idle
                  # gap collapses the clock and re-ramping costs ~5us, so
                  # overshooting by a few dummies is far cheaper than a gap

_NC_CACHE = {}


def _build():
    import concourse.bacc as bacc
    import concourse.mybir as mybir
    from concourse.tile import TileContext
    from concourse.bass import ts

    f32 = mybir.dt.float32
    f16 = mybir.dt.float16
    f8 = mybir.dt.float8e4
    AF = mybir.ActivationFunctionType
    DR = mybir.MatmulPerfMode.DoubleRow

    nc = bacc.Bacc()
    fb_d = nc.declare_dram_parameter("fb", [128, FB_W], f16, isOutput=False)
    # partition-major fp8 edge: pairs 0-6 as [p, pair, t, j]; pair 7
    # chunk-major [p, c, t, j'] for the 4 tail DMAs
    edge_d = nc.declare_dram_parameter("edge", [128, NPAIR * 2 * M], f8, isOutput=False)
    bb_d = nc.declare_dram_parameter("bb", [128, BB_W], f32, isOutput=False)
    outT_d = nc.declare_dram_parameter("outT", [D_HID, M], f16, isOutput=True)

    with TileContext(nc) as tc:
        with (
            tc.tile_pool(name="const", bufs=1) as cpool,
            tc.tile_pool(name="acts", bufs=1) as apool,
            tc.tile_pool(name="edgep", bufs=NPAIR) as epool,
            tc.tile_pool(name="pout", bufs=1, space="PSUM") as pout_pool,
            tc.tile_pool(name="pg", bufs=2, space="PSUM") as pg_pool,
            tc.tile_pool(name="pwork", bufs=2, space="PSUM") as pwork_pool,
        ):
            fbh = cpool.tile([128, FW_W + 512], f16, name="fbh")  # weights + xT0
            fbx1 = cpool.tile([128, 512], f16, name="fbx1")       # xT1
            bb = cpool.tile([128, 1, BB_W], f32, name="bb")

            # ---- all input DMAs on the Sync ring, fb/bb first: the MLP
            # must start the instant the warm-up dummies end (a PE idle gap
            # collapses the clock, and re-ramping costs ~5us), so fb's
            # arrival time bounds everything. The edge stream owns the ring
            # afterwards.
            nc.sync.dma_start(out=fbh, in_=fb_d[:, 0:FW_W + 512])
            nc.sync.dma_start(out=bb, in_=bb_d[:])
            nc.sync.dma_start(out=fbx1, in_=fb_d[:, FW_W + 512:FB_W])
            ed0 = epool.tile([128, 2, M], f8, tag="e", name="ed0")
            nc.sync.dma_start(out=ed0, in_=edge_d[:, 0:2 * M])
            edd = [
                epool.tile([128, 2, 2, M], f8, tag="e", name=f"edd{d}")
                for d in range(2)
            ]
            for d in range(2):
                nc.sync.dma_start(
                    out=edd[d],
                    in_=edge_d[:, (1 + 2 * d) * 2 * M:(3 + 2 * d) * 2 * M],
                )
            ed5 = epool.tile([128, 2, M], f8, tag="e", name="ed5")
            nc.sync.dma_start(out=ed5, in_=edge_d[:, 5 * 2 * M:6 * 2 * M])
            ed6 = epool.tile([128, 2, M], f8, tag="e", name="ed6")
            nc.sync.dma_start(out=ed6, in_=edge_d[:, 6 * 2 * M:7 * 2 * M])
            eds = [ed0, edd[0][:, 0], edd[0][:, 1], edd[1][:, 0], edd[1][:, 1], ed5, ed6]
            ed7 = [
                epool.tile([128, 2, 512], f8, tag="e", name=f"ed7c{c}")
                for c in range(NCH)
            ]
            for c in range(NCH):
                nc.sync.dma_start(
                    out=ed7[c],
                    in_=edge_d[:, (NPAIR - 1) * 2 * M + c * 1024:(NPAIR - 1) * 2 * M + (c + 1) * 1024],
                )

            w_g2 = fbh[0:128, _W_GW2:_W_GW2 + 128]
            w_f2 = fbh[0:64, _W_FW2:_W_FW2 + 128]
            gb2b2 = bb[0:128, 0:1, _B_GB2:_B_GB2 + 128].broadcast_to([128, 2, 128])
            b_f1 = bb[0:64, 0, _B_F1:_B_F1 + 1]
            b_g1 = bb[0:128, 0, _B_G1:_B_G1 + 1]
            b_f2 = bb[0:128, 0, _B_F2:_B_F2 + 1]

            h1f = apool.tile([D_IN, M], f16, name="h1f")
            h1g = apool.tile([D_HID, M], f16, name="h1g")
            gx = apool.tile([128, NT, 128], f8, name="gx")  # tile i: [t, h] node-major
            # two store tiles: store A covers chunks 0-1, B covers 2-3, so
            # each 1024-col store DMA fires as soon as its two biases land
            outTab = [apool.tile([128, 1024], f16, name=f"outT{h}") for h in range(2)]
            pouts = [pout_pool.tile([128, 512], f32, name=f"pout{c}") for c in range(NCH)]

            # warm the ACT function table during the preamble (hoists the
            # lazy ~1.4us ACT_TABLE_LOAD off the h1g critical path)
            warm = apool.tile([1, 1], f32, name="warm")
            scratch = apool.tile([128, 256], f16, name="scratch")
            nc.gpsimd.memset(scratch, 0)
            nc.scalar.activation(warm, scratch[0:1, 0:1], AF.Relu, bias=0.0)

            # PE warm-up: the tensor engine clock-gates to ~half speed unless
            # continuously busy for several us; dummies bridge until fbh lands
            for _ in range(N_DUMMIES):
                psd = pg_pool.tile([128, 256], f32, tag="g", name="psd")
                nc.tensor.matmul(psd, scratch[:, 0:128], scratch, start=True, stop=True)

            # token-chunk c (tokens 512c..512c+512): xT half c%2, rows 64*(c//2)
            def tok_src(c):
                a, c2 = divmod(c, 2)
                src = fbh[:, _F_XT0:_F_XT0 + 512] if c2 == 0 else fbx1
                return src[64 * a:64 * a + 64, :]

            def rows_of(c):
                a = c // 2
                return slice(64 * a, 64 * a + 64)

            def h1g_chunk(c):
                rows = rows_of(c)
                w_g1 = fbh[rows, _W_WG1:_W_WG1 + 128]
                psg = pg_pool.tile([128, 512], f32, tag="g", name="psg")
                nc.tensor.matmul(psg, w_g1, tok_src(c), start=True, stop=True)
                nc.scalar.activation(h1g[:, ts(c, 512)], psg, AF.Relu, bias=b_g1)

            def h1f_chunk(c):
                # relu+bias on Scalar (GpSimd can't read PSUM; Vector stays
                # lean for the gx bias-adds on the DoubleRow critical path)
                rows = rows_of(c)
                w_f1 = fbh[rows, _W_FW1:_W_FW1 + 64]
                psf = pwork_pool.tile([64, 512], f32, tag="w", name="psf")
                nc.tensor.matmul(psf, w_f1, tok_src(c), start=True, stop=True)
                nc.scalar.activation(h1f[:, ts(c, 512)], psf, AF.Relu, bias=b_f1)

            def gx_pair(p):
                # gx tiles 2p, 2p+1 (node-major [t, h]): 2 matmuls into one
                # PSUM tile, one short DVE bias-add with fp8 downcast
                psx = pwork_pool.tile([128, 256], f32, tag="w", name="psx")
                for k in range(2):
                    i = 2 * p + k
                    nc.tensor.matmul(
                        psx[:, ts(k, 128)], h1g[:, ts(i, 128)], w_g2,
                        start=True, stop=True,
                    )
                nc.vector.tensor_add(gx[:, 2 * p:2 * p + 2, :], psx, gb2b2)

            def dr_pair(p, start=False, stop=False):
                lhsT = gx[:, 2 * p:2 * p + 2, :]
                for c in range(NCH):
                    rhs = ed7[c] if p == NPAIR - 1 else eds[p][:, :, ts(c, 512)]
                    nc.tensor.matmul(
                        pouts[c], lhsT, rhs,
                        start=start, stop=stop,
                        perf_mode=DR,
                    )

            # ---- dense MLP phase (all fb landed before the dummies end, so
            # no data stalls break the clock ramp), then the DoubleRow
            # stream paced by the edge DMA arrivals. The self-dyn opens fill
            # the h1g1-relu latency before gx pairs 2-3; gx pairs 6-7 slot
            # between the first DR pairs to cover the h1g3-relu latency.
            h1f_chunk(0)
            h1f_chunk(1)
            h1g_chunk(0)
            h1f_chunk(2)
            h1f_chunk(3)
            gx_pair(0)
            gx_pair(1)
            h1g_chunk(1)
            # self-dynamics opens each output chunk's PSUM accumulation group
            for c in range(NCH):
                nc.tensor.matmul(
                    pouts[c], w_f2, h1f[:, ts(c, 512)],
                    start=True, stop=False,
                )
            gx_pair(2)
            gx_pair(3)
            h1g_chunk(2)
            gx_pair(4)
            gx_pair(5)
            h1g_chunk(3)
            dr_pair(0)
            gx_pair(6)
            gx_pair(7)
            for p in range(1, NPAIR):
                dr_pair(p, stop=(p == NPAIR - 1))

            # tail: per-chunk bias-add split into ACT/DVE halves (parallel,
            # halves the bias latency); two 1024-col stores on the Sync ring
            # (2KB packets move ~2-3x faster than 1KB ones)
            for c in range(NCH):
                dst = outTab[c // 2][:, ts(c % 2, 512)]
                nc.scalar.activation(
                    dst[:, 0:256], pouts[c][:, 0:256], AF.Identity, bias=b_f2
                )
                nc.vector.tensor_scalar_add(
                    dst[:, 256:512], pouts[c][:, 256:512], b_f2
                )
                if c % 2 == 1:
                    nc.sync.dma_start(
                        out=outT_d[:, ts(c // 2, 1024)], in_=outTab[c // 2]
                    )
    nc.compile()
    return nc


def _get_nc():
    if "nc" not in _NC_CACHE:
        _NC_CACHE["nc"] = _build()
    return _NC_CACHE["nc"]


def _prep_in_maps(inputs):
    import ml_dtypes

    x = np.asarray(inputs["x"], dtype=np.float32)
    edge = np.asarray(inputs["edge"], dtype=np.float32)
    f_w1 = np.asarray(inputs["f_w1"], dtype=np.float32)
    f_b1 = np.asarray(inputs["f_b1"], dtype=np.float32)
    f_w2 = np.asarray(inputs["f_w2"], dtype=np.float32)
    f_b2 = np.asarray(inputs["f_b2"], dtype=np.float32)
    g_w1 = np.asarray(inputs["g_w1"], dtype=np.float32)
    g_b1 = np.asarray(inputs["g_b1"], dtype=np.float32)
    g_w2 = np.asarray(inputs["g_w2"], dtype=np.float32)
    g_b2 = np.asarray(inputs["g_b2"], dtype=np.float32)

    # cat(x, x) @ g_w1 == x @ (g_w1[:64] + g_w1[64:])
    wg1 = g_w1[:D_IN] + g_w1[D_IN:]

    # x[n].T packed [128, 1024]: xT2[64a + k, t] = x[n, 1024a + t, k]
    xT = np.transpose(x, (0, 2, 1)).astype(np.float16)       # [8, 64, 2048]
    xT2 = np.concatenate([xT[:, :, :1024], xT[:, :, 1024:]], axis=1)  # [8, 128, 1024]

    fb = np.zeros((N_BATCH, 128, FB_W), dtype=np.float16)
    for r in (slice(0, 64), slice(64, 128)):  # duplicate for partition-64 rhs
        fb[:, r, _W_FW1:_W_FW1 + 64] = f_w1.astype(np.float16)
        fb[:, r, _W_FW2:_W_FW2 + 128] = f_w2.astype(np.float16)
        fb[:, r, _W_WG1:_W_WG1 + 128] = wg1.astype(np.float16)
    fb[:, 0:128, _W_GW2:_W_GW2 + 128] = g_w2.astype(np.float16)
    fb[:, :, _F_XT0:_F_XT0 + 512] = xT2[:, :, 0:512]
    fb[:, :, _F_XT1:_F_XT1 + 512] = xT2[:, :, 512:1024]
    fb = np.ascontiguousarray(fb)

    bb = np.zeros((128, BB_W), dtype=np.float32)
    bb[0:128, _B_GB2:_B_GB2 + 128] = g_b2[None, :]
    bb[0:64, _B_F1] = f_b1
    bb[0:128, _B_G1] = g_b1
    bb[0:128, _B_F2] = f_b2

    # edge fp8 partition-major: edge_r[n][p, pair, t, j] = edge8[n, 128*(2*pair+t)+p, j]
    # except pair 7, which is chunk-major [p, c, t, j'] for the 4 tail DMAs
    edge8 = edge.astype(ml_dtypes.float8_e4m3)               # [8, 2048, 2048]
    edge_r = (
        edge8.reshape(N_BATCH, NPAIR, 2, 128, M)
        .transpose(0, 3, 1, 2, 4)
        .reshape(N_BATCH, 128, NPAIR, 2 * M)
        .copy()
    )
    p7 = (
        edge8[:, (NPAIR - 1) * 256:, :]                      # [8, 256, 2048]
        .reshape(N_BATCH, 2, 128, NCH, 512)
        .transpose(0, 2, 3, 1, 4)                            # [8, 128, c, t, 512]
        .reshape(N_BATCH, 128, 2 * M)
    )
    edge_r[:, :, NPAIR - 1, :] = p7
    edge_r = np.ascontiguousarray(edge_r).reshape(N_BATCH, 128, NPAIR * 2 * M)

    in_maps = [
        {
            "fb": fb[n],
            "edge": edge_r[n],
            "bb": bb,
        }
        for n in range(N_BATCH)
    ]
    return in_maps


def run(inputs, trace=False, **kw):
    """Run on 8 cores; returns (out [8, 2048, 128] fp32, BassKernelResults)."""
    from concourse.bass_utils import run_bass_kernel_spmd

    nc = _get_nc()
    in_maps = _prep_in_maps(inputs)
    res = run_bass_kernel_spmd(nc, in_maps, list(range(NCORES)), trace=trace, **kw)
    outT = np.stack([np.asarray(res.results[n]["outT"]) for n in range(N_BATCH)])
    out = np.ascontiguousarray(np.transpose(outT, (0, 2, 1)))  # [8, 2048, 128]
    return out.astype(np.float32), res


def kernel(**inputs):
    out, _ = run(inputs, trace=False)
    return out


# revision 19
# speedup vs baseline: 1.1421x; 1.0687x over previous
"""Trainium2 Bass kernel for nn_Node_GCN: out[n] = f(x[n]) + edge[n]^T @ g(cat(x,x)[n]).

Sharding: data-parallel over the batch dim N=8, one batch per NeuronCore.

edge is carried in fp8(e4m3) — halves the dominant HBM stream vs fp16 — and
gx is quantized to fp8 so the 2048x2048x128 edge contraction runs in
MatmulPerfMode.DoubleRow (216ns issue per 512-col matmul = the fp8 PE
roofline). Quantization error lands at rel_l2 ~1.8e-3 (vs 2e-2 budget).

DMA facts this schedule is built on (measured from ntff traces):
- All DGE rings share the 16 engines, but arbitration is per-packet, so a
  queue with small packets starves while the edge stream runs. Everything
  bandwidth-critical rides ONE ring (Sync), which round-robins across its
  in-flight DMAs at ~350-425 B/ns aggregate with 4KB packets.
- A DMA post costs ~0.7us of queue time; packets start ~0.8us later.
- The PE clock-gates to ~half speed unless continuously busy for several
  us (warm-up dummies bridge until the first data lands).

Order on the Sync ring: fb head (weights + xT half 0), bb, fb tail, then
the edge pairs smallest-granularity-last (pair 0 single, 1-4 as two 2-pair
DMAs, 5 and 6 single, pair 7 as 4 chunk DMAs so the final DoubleRow + bias
+ store chain hangs off 0.13MB). Output stores are two 1024-col DMAs (2KB
packets) on the same ring.

The device computes outT[n] = [h, j] fp16; the host transposes to [j, h]
and widens to fp32 while unsharding.
"""

import numpy as np

D_IN = 64
D_HID = 128
M = 2048          # nodes per batch
N_BATCH = 8
NCORES = 8

NT = M // 128     # 16 sender k-tiles
NPAIR = NT // 2   # 8 DoubleRow pairs
NCH = M // 512    # 4 output chunks of 512

# fp16 blob [128, FB_W]: weights first, then xT halves
_W_FW1 = 0          # f_w1 [64, 64]
_W_FW2 = 64         # f_w2 [64, 128]
_W_WG1 = 192        # wg1  [64, 128]  (= g_w1[:64] + g_w1[64:])
_W_GW2 = 320        # g_w2 [128, 128]
FW_W = 448
_F_XT0 = FW_W       # xT cols 0:512   (both 64-row halves)
_F_XT1 = FW_W + 512 # xT cols 512:1024
FB_W = FW_W + 1024

# fp32 bias blob [128, BB_W]
_B_GB2 = 0          # g_b2 [128, 128], broadcast 2x along free at use site
_B_F1 = 128         # f_b1 [64, 1]
_B_G1 = 129         # g_b1 [128, 1]
_B_F2 = 130         # f_b2 [128, 1]
BB_W = 131

N_DUMMIES = 25    # PE warm-up matmuls (213ns each): sized to end just after
                  # the whole fb blob lands (~12.4us; the DGE ring round-
                  # robins fb with the edge stream, so it can't land
                  # earlier) — a PE idle gap collapses the clock and
                  # re-ramping costs ~5us, so overshooting is far cheaper

_NC_CACHE = {}


def _build():
    import concourse.bacc as bacc
    import concourse.mybir as mybir
    from concourse.tile import TileContext
    from concourse.bass import ts

    f32 = mybir.dt.float32
    f16 = mybir.dt.float16
    f8 = mybir.dt.float8e4
    AF = mybir.ActivationFunctionType
    DR = mybir.MatmulPerfMode.DoubleRow

    nc = bacc.Bacc()
    fb_d = nc.declare_dram_parameter("fb", [128, FB_W], f16, isOutput=False)
    # partition-major fp8 edge: pairs 0-6 as [p, pair, t, j]; pair 7
    # chunk-major [p, c, t, j'] for the 4 tail DMAs
    edge_d = nc.declare_dram_parameter("edge", [128, NPAIR * 2 * M], f8, isOutput=False)
    bb_d = nc.declare_dram_parameter("bb", [128, BB_W], f32, isOutput=False)
    outT_d = nc.declare_dram_parameter("outT", [D_HID, M], f16, isOutput=True)

    with TileContext(nc) as tc:
        with (
            tc.tile_pool(name="const", bufs=1) as cpool,
            tc.tile_pool(name="acts", bufs=1) as apool,
            tc.tile_pool(name="edgep", bufs=NPAIR) as epool,
            tc.tile_pool(name="pout", bufs=1, space="PSUM") as pout_pool,
            tc.tile_pool(name="pg", bufs=2, space="PSUM") as pg_pool,
            tc.tile_pool(name="pwork", bufs=2, space="PSUM") as pwork_pool,
        ):
            fbh = cpool.tile([128, FW_W + 512], f16, name="fbh")  # weights + xT0
            fbx1 = cpool.tile([128, 512], f16, name="fbx1")       # xT1
            bb = cpool.tile([128, 1, BB_W], f32, name="bb")

            # ---- all input DMAs on the Sync ring, fb/bb first: the MLP
            # must start the instant the warm-up dummies end (a PE idle gap
            # collapses the clock, and re-ramping costs ~5us), so fb's
            # arrival time bounds everything. The edge stream owns the ring
            # afterwards.
            nc.sync.dma_start(out=bb, in_=bb_d[:])
            nc.sync.dma_start(out=fbh, in_=fb_d[:, 0:FW_W + 512])
            nc.sync.dma_start(out=fbx1, in_=fb_d[:, FW_W + 512:FB_W])
            ed0 = epool.tile([128, 2, M], f8, tag="e", name="ed0")
            nc.sync.dma_start(out=ed0, in_=edge_d[:, 0:2 * M])
            edd = [
                epool.tile([128, 2, 2, M], f8, tag="e", name=f"edd{d}")
                for d in range(2)
            ]
            for d in range(2):
                nc.sync.dma_start(
                    out=edd[d],
                    in_=edge_d[:, (1 + 2 * d) * 2 * M:(3 + 2 * d) * 2 * M],
                )
            ed5 = epool.tile([128, 2, M], f8, tag="e", name="ed5")
            nc.sync.dma_start(out=ed5, in_=edge_d[:, 5 * 2 * M:6 * 2 * M])
            ed6 = epool.tile([128, 2, M], f8, tag="e", name="ed6")
            nc.sync.dma_start(out=ed6, in_=edge_d[:, 6 * 2 * M:7 * 2 * M])
            eds = [ed0, edd[0][:, 0], edd[0][:, 1], edd[1][:, 0], edd[1][:, 1], ed5, ed6]
            ed7 = [
                epool.tile([128, 2, 512], f8, tag="e", name=f"ed7c{c}")
                for c in range(NCH)
            ]
            for c in range(NCH):
                nc.sync.dma_start(
                    out=ed7[c],
                    in_=edge_d[:, (NPAIR - 1) * 2 * M + c * 1024:(NPAIR - 1) * 2 * M + (c + 1) * 1024],
                )

            w_g2 = fbh[0:128, _W_GW2:_W_GW2 + 128]
            w_f2 = fbh[0:64, _W_FW2:_W_FW2 + 128]
            gb2b2 = bb[0:128, 0:1, _B_GB2:_B_GB2 + 128].broadcast_to([128, 2, 128])
            b_f1 = bb[0:64, 0, _B_F1:_B_F1 + 1]
            b_g1 = bb[0:128, 0, _B_G1:_B_G1 + 1]
            b_f2 = bb[0:128, 0, _B_F2:_B_F2 + 1]

            h1f = apool.tile([D_IN, M], f16, name="h1f")
            h1g = apool.tile([D_HID, M], f16, name="h1g")
            gx = apool.tile([128, NT, 128], f8, name="gx")  # tile i: [t, h] node-major
            # two store tiles: store A covers chunks 0-1, B covers 2-3, so
            # each 1024-col store DMA fires as soon as its two biases land
            outTab = [apool.tile([128, 1024], f16, name=f"outT{h}") for h in range(2)]
            pouts = [pout_pool.tile([128, 512], f32, name=f"pout{c}") for c in range(NCH)]

            # warm the ACT function table during the preamble (hoists the
            # lazy ~1.4us ACT_TABLE_LOAD off the h1g critical path)
            warm = apool.tile([1, 1], f32, name="warm")
            scratch = apool.tile([128, 256], f16, name="scratch")
            nc.gpsimd.memset(scratch, 0)
            nc.scalar.activation(warm, scratch[0:1, 0:1], AF.Relu, bias=0.0)

            # PE warm-up: the tensor engine clock-gates to ~half speed unless
            # continuously busy for several us; dummies bridge until fbh lands
            for _ in range(N_DUMMIES):
                psd = pg_pool.tile([128, 256], f32, tag="g", name="psd")
                nc.tensor.matmul(psd, scratch[:, 0:128], scratch, start=True, stop=True)

            # token-chunk c (tokens 512c..512c+512): xT half c%2, rows 64*(c//2)
            def tok_src(c):
                a, c2 = divmod(c, 2)
                src = fbh[:, _F_XT0:_F_XT0 + 512] if c2 == 0 else fbx1
                return src[64 * a:64 * a + 64, :]

            def rows_of(c):
                a = c // 2
                return slice(64 * a, 64 * a + 64)

            def h1g_chunk(c):
                rows = rows_of(c)
                w_g1 = fbh[rows, _W_WG1:_W_WG1 + 128]
                psg = pg_pool.tile([128, 512], f32, tag="g", name="psg")
                nc.tensor.matmul(psg, w_g1, tok_src(c), start=True, stop=True)
                nc.scalar.activation(h1g[:, ts(c, 512)], psg, AF.Relu, bias=b_g1)

            def h1f_chunk(c):
                # relu+bias on Scalar (GpSimd can't read PSUM; Vector stays
                # lean for the gx bias-adds on the DoubleRow critical path)
                rows = rows_of(c)
                w_f1 = fbh[rows, _W_FW1:_W_FW1 + 64]
                psf = pwork_pool.tile([64, 512], f32, tag="w", name="psf")
                nc.tensor.matmul(psf, w_f1, tok_src(c), start=True, stop=True)
                nc.scalar.activation(h1f[:, ts(c, 512)], psf, AF.Relu, bias=b_f1)

            def gx_pair(p):
                # gx tiles 2p, 2p+1 (node-major [t, h]): 2 matmuls into one
                # PSUM tile, one short DVE bias-add with fp8 downcast
                psx = pwork_pool.tile([128, 256], f32, tag="w", name="psx")
                for k in range(2):
                    i = 2 * p + k
                    nc.tensor.matmul(
                        psx[:, ts(k, 128)], h1g[:, ts(i, 128)], w_g2,
                        start=True, stop=True,
                    )
                nc.vector.tensor_add(gx[:, 2 * p:2 * p + 2, :], psx, gb2b2)

            def dr_pair(p, start=False, stop=False):
                lhsT = gx[:, 2 * p:2 * p + 2, :]
                for c in range(NCH):
                    rhs = ed7[c] if p == NPAIR - 1 else eds[p][:, :, ts(c, 512)]
                    nc.tensor.matmul(
                        pouts[c], lhsT, rhs,
                        start=start, stop=stop,
                        perf_mode=DR,
                    )

            # ---- dense MLP phase (all fb landed before the dummies end, so
            # no data stalls break the clock ramp), then the DoubleRow
            # stream paced by the edge DMA arrivals. The self-dyn opens fill
            # the h1g1-relu latency before gx pairs 2-3; gx pairs 6-7 slot
            # between the first DR pairs to cover the h1g3-relu latency.
            h1f_chunk(0)
            h1f_chunk(1)
            h1g_chunk(0)
            h1f_chunk(2)
            h1f_chunk(3)
            gx_pair(0)
            gx_pair(1)
            h1g_chunk(1)
            # self-dynamics opens each output chunk's PSUM accumulation group
            for c in range(NCH):
                nc.tensor.matmul(
                    pouts[c], w_f2, h1f[:, ts(c, 512)],
                    start=True, stop=False,
                )
            gx_pair(2)
            gx_pair(3)
            h1g_chunk(2)
            gx_pair(4)
            gx_pair(5)
            h1g_chunk(3)
            dr_pair(0)
            gx_pair(6)
            gx_pair(7)
            for p in range(1, NPAIR):
                dr_pair(p, stop=(p == NPAIR - 1))

            # tail: per-chunk bias-add split into ACT/DVE halves (parallel,
            # halves the bias latency); two 1024-col stores on the Sync ring
            # (2KB packets move ~2-3x faster than 1KB ones)
            for c in range(NCH):
                dst = outTab[c // 2][:, ts(c % 2, 512)]
                nc.scalar.activation(
                    dst[:, 0:256], pouts[c][:, 0:256], AF.Identity, bias=b_f2
                )
                nc.vector.tensor_scalar_add(
                    dst[:, 256:512], pouts[c][:, 256:512], b_f2
                )
                if c % 2 == 1:
                    nc.sync.dma_start(
                        out=outT_d[:, ts(c // 2, 1024)], in_=outTab[c // 2]
                    )
    nc.compile()
    return nc


def _get_nc():
    if "nc" not in _NC_CACHE:
        _NC_CACHE["nc"] = _build()
    return _NC_CACHE["nc"]


def _prep_in_maps(inputs):
    import ml_dtypes

    x = np.asarray(inputs["x"], dtype=np.float32)
    edge = np.asarray(inputs["edge"], dtype=np.float32)
    f_w1 = np.asarray(inputs["f_w1"], dtype=np.float32)
    f_b1 = np.asarray(inputs["f_b1"], dtype=np.float32)
    f_w2 = np.asarray(inputs["f_w2"], dtype=np.float32)
    f_b2 = np.asarray(inputs["f_b2"], dtype=np.float32)
    g_w1 = np.asarray(inputs["g_w1"], dtype=np.float32)
    g_b1 = np.asarray(inputs["g_b1"], dtype=np.float32)
    g_w2 = np.asarray(inputs["g_w2"], dtype=np.float32)
    g_b2 = np.asarray(inputs["g_b2"], dtype=np.float32)

    # cat(x, x) @ g_w1 == x @ (g_w1[:64] + g_w1[64:])
    wg1 = g_w1[:D_IN] + g_w1[D_IN:]

    # x[n].T packed [128, 1024]: xT2[64a + k, t] = x[n, 1024a + t, k]
    xT = np.transpose(x, (0, 2, 1)).astype(np.float16)       # [8, 64, 2048]
    xT2 = np.concatenate([xT[:, :, :1024], xT[:, :, 1024:]], axis=1)  # [8, 128, 1024]

    fb = np.zeros((N_BATCH, 128, FB_W), dtype=np.float16)
    for r in (slice(0, 64), slice(64, 128)):  # duplicate for partition-64 rhs
        fb[:, r, _W_FW1:_W_FW1 + 64] = f_w1.astype(np.float16)
        fb[:, r, _W_FW2:_W_FW2 + 128] = f_w2.astype(np.float16)
        fb[:, r, _W_WG1:_W_WG1 + 128] = wg1.astype(np.float16)
    fb[:, 0:128, _W_GW2:_W_GW2 + 128] = g_w2.astype(np.float16)
    fb[:, :, _F_XT0:_F_XT0 + 512] = xT2[:, :, 0:512]
    fb[:, :, _F_XT1:_F_XT1 + 512] = xT2[:, :, 512:1024]
    fb = np.ascontiguousarray(fb)

    bb = np.zeros((128, BB_W), dtype=np.float32)
    bb[0:128, _B_GB2:_B_GB2 + 128] = g_b2[None, :]
    bb[0:64, _B_F1] = f_b1
    bb[0:128, _B_G1] = g_b1
    bb[0:128, _B_F2] = f_b2

    # edge fp8 partition-major: edge_r[n][p, pair, t, j] = edge8[n, 128*(2*pair+t)+p, j]
    # except pair 7, which is chunk-major [p, c, t, j'] for the 4 tail DMAs
    edge8 = edge.astype(ml_dtypes.float8_e4m3)               # [8, 2048, 2048]
    edge_r = (
        edge8.reshape(N_BATCH, NPAIR, 2, 128, M)
        .transpose(0, 3, 1, 2, 4)
        .reshape(N_BATCH, 128, NPAIR, 2 * M)
        .copy()
    )
    p7 = (
        edge8[:, (NPAIR - 1) * 256:, :]                      # [8, 256, 2048]
        .reshape(N_BATCH, 2, 128, NCH, 512)
        .transpose(0, 2, 3, 1, 4)                            # [8, 128, c, t, 512]
        .reshape(N_BATCH, 128, 2 * M)
    )
    edge_r[:, :, NPAIR - 1, :] = p7
    edge_r = np.ascontiguousarray(edge_r).reshape(N_BATCH, 128, NPAIR * 2 * M)

    in_maps = [
        {
            "fb": fb[n],
            "edge": edge_r[n],
            "bb": bb,
        }
        for n in range(N_BATCH)
    ]
    return in_maps


def run(inputs, trace=False, **kw):
    """Run on 8 cores; returns (out [8, 2048, 128] fp32, BassKernelResults)."""
    from concourse.bass_utils import run_bass_kernel_spmd

    nc = _get_nc()
    in_maps = _prep_in_maps(inputs)
    res = run_bass_kernel_spmd(nc, in_maps, list(range(NCORES)), trace=trace, **kw)
    outT = np.stack([np.asarray(res.results[n]["outT"]) for n in range(N_BATCH)])
    out = np.ascontiguousarray(np.transpose(outT, (0, 2, 1)))  # [8, 2048, 128]
    return out.astype(np.float32), res


def kernel(**inputs):
    out, _ = run(inputs, trace=False)
    return out
